# revision 1
# baseline (speedup 1.0000x reference)
"""Causal self-attention Trainium2 kernel (B=4, T=2048, C=1024, H=16, D=64).

Sharding: 8 cores = 4 batches x 2 causally-balanced query shards.
Core c handles batch b=c//2 and the 8 interleaved query blocks
g = 2*i + (c%2), i in 0..7 (block = 128 rows).  Every core computes full-
sequence K/V for its batch plus Q for its own query rows, runs all 16 heads
of attention for those rows, and the full output projection for them, so
per-core outputs are disjoint row-slices of y: no collectives, no host math.

Device-side dataflow (all matmuls fp16 in / fp32 PSUM accumulate):
  qkv^T = w^T @ x^T        (x^T supplied by host; K^T/Q^T land in [d, t]
                            layout with head pairs stacked on partitions)
  S^T   = K @ Q^T          (two heads row-tiled on the PE array, contract=64)
  P^T   = exp(0.125*S^T)   (ACT, psum->sbuf, fp16 out) then 0/1 mask mult
  [Y|s] = P^T.T @ [V|1]    (ones-augmented V gives softmax sums in col 64)
  Yn    = Y * (1/s)        (per-partition scalar on DVE)
  Y^T   via PE transpose   (feeds out-proj as lhsT)
  Z     = Y^T.T @ w_out + ones x b_out  (bias as a K=1 matmul)
"""

import math
import numpy as np

B, T, C = 4, 2048, 1024
H, D = 16, 64
N_CORES = 8
P = 128
QB = 8  # local query blocks per core (of 128 rows)
KB = 16  # key blocks per sequence
PAIRS = 8  # head pairs

_COMPILED = None
LAST_EXEC_NS = None


def _get_mybir():
    import concourse.mybir as mybir
    return mybir


def split_sync_waits(nc):
    mybir = _get_mybir()
    # This walrus build rejects instructions carrying more than one sync
    # wait (or update).  Split the extras onto NOP carriers: waits go on
    # NOPs inserted before the instruction (same engine, so they gate it),
    # updates onto NOPs after it (fire once it has completed).
    uid = [0]

    def carrier(engine, wait=None, update=None):
        uid[0] += 1
        n = mybir.InstNoOp(
            name=f"I-syncsplit-{uid[0]}",
            opcode="NoOp",
            ins=[],
            outs=[],
            sync_info=mybir.SyncInfo(
                on_wait=[wait] if wait else [],
                on_update=[update] if update else [],
            ),
        )
        n.engine = engine
        return n

    import os

    debug = os.environ.get("SYNC_SPLIT_DEBUG")
    for f in nc.m.functions:
        for blk in f.blocks:
            out = []
            changed = False
            for inst in blk.instructions:
                si = inst.sync_info
                if si is None or (
                    len(si.on_wait) <= 1 and len(si.on_update) <= 1
                ):
                    out.append(inst)
                    continue
                if debug:
                    print(
                        f"SPLIT {inst.opcode} {inst.name} eng={inst.engine} "
                        f"waits={len(si.on_wait)} upds={len(si.on_update)}"
                    )
                changed = True
                waits = list(si.on_wait)
                updates = list(si.on_update)
                for w in waits[1:]:
                    out.append(carrier(inst.engine, wait=w))
                inst.sync_info = mybir.SyncInfo(
                    on_wait=waits[:1], on_update=updates[:1]
                )
                out.append(inst)
                for u in updates[1:]:
                    out.append(carrier(inst.engine, update=u))
            if changed:
                blk.instructions = out

import os

dbg_stage = os.environ.get("KERNEL_DEBUG_STAGE", "")




def _build():
    import concourse.bass as bass
    import concourse.tile as tile
    import concourse.mybir as mybir
    from concourse.vector_clock import ScopedClock
    from contextlib import ExitStack

    f32 = mybir.dt.float32
    f16 = mybir.dt.float16
    AF = mybir.ActivationFunctionType

    nc = bass.Bass(
        "TRN2", target_bir_lowering=False, debug=False, num_devices=N_CORES
    )

    xT_d = nc.dram_tensor("xt", [C, T], f16, kind="ExternalInput").ap()
    xqT_d = nc.dram_tensor("xqt", [C, QB * P], f16, kind="ExternalInput").ap()
    wqkv_d = nc.dram_tensor("wqkv", [24, P, 8, P], f16, kind="ExternalInput").ap()
    wout_d = nc.dram_tensor("wout", [P, 8, C], f16, kind="ExternalInput").ap()
    bqkv_d = nc.dram_tensor("bqkv", [P, 24], f32, kind="ExternalInput").ap()
    bout_d = nc.dram_tensor("bout", [1, C], f16, kind="ExternalInput").ap()
    mask_d = nc.dram_tensor("mask", [P, 2, P], f16, kind="ExternalInput").ap()
    id64_d = nc.dram_tensor("id64", [P, 64], f16, kind="ExternalInput").ap()
    id128_d = nc.dram_tensor("id128", [P, P], f16, kind="ExternalInput").ap()
    ones_d = nc.dram_tensor("ones1", [1, P], f16, kind="ExternalInput").ap()
    yr_d = nc.dram_tensor("yr", [P, QB, C], f32, kind="ExternalOutput").ap()
    dbg_d = {}
    if dbg_stage in ("proj", "vn", "attn"):
        dbg_d["kt"] = nc.dram_tensor(
            "kt", [P, PAIRS, T], f16, kind="ExternalOutput"
        ).ap()
        dbg_d["qt"] = nc.dram_tensor(
            "qt", [P, PAIRS, QB * P], f16, kind="ExternalOutput"
        ).ap()
    if dbg_stage in ("vn", "attn"):
        dbg_d["vn"] = nc.dram_tensor(
            "vn", [P, PAIRS, KB, 2, 65], f16, kind="ExternalOutput"
        ).ap()
    if dbg_stage == "attn":
        dbg_d["yt"] = nc.dram_tensor(
            "yt", [P, 8, QB * P], f16, kind="ExternalOutput"
        ).ap()

    with tile.TileContext(nc) as tc, ExitStack() as ctx:
        persist = ctx.enter_context(tc.tile_pool(name="persist", bufs=1))
        KT = persist.tile([P, PAIRS, T], f16)  # K^T + bias, head pairs stacked
        QT = persist.tile([P, PAIRS, QB * P], f16)
        YT = persist.tile([P, 8, QB * P], f16)  # attention out, [c, tq]
        WO = persist.tile([P, 8, C], f16)
        msk = persist.tile([P, 2, P], f16)
        bqs = persist.tile([P, 24], f32)
        i64 = persist.tile([P, 64], f16)
        i128 = persist.tile([P, P], f16)
        on1 = persist.tile([1, P], f16)
        bo = persist.tile([1, C], f16)

        nc.sync.dma_start(out=msk, in_=mask_d)
        nc.sync.dma_start(out=bqs, in_=bqkv_d)
        nc.sync.dma_start(out=i64, in_=id64_d)
        nc.sync.dma_start(out=i128, in_=id128_d)
        nc.sync.dma_start(out=on1, in_=ones_d)
        nc.sync.dma_start(out=bo, in_=bout_d)
        nc.sync.dma_start(out=WO, in_=wout_d)

        wpool = ctx.enter_context(tc.tile_pool(name="w", bufs=3))
        vtpool = ctx.enter_context(tc.tile_pool(name="vt", bufs=1))
        VT = vtpool.tile([P, PAIRS, T], f16)

        xT_v = xT_d.rearrange("(cb p) t -> p cb t", p=P)
        xqT_v = xqT_d.rearrange("(cb p) t -> p cb t", p=P)

        with (
            tc.tile_pool(name="xt", bufs=1) as xtpool,
            tc.tile_pool(name="psproj", bufs=3, space="PSUM") as pspool,
        ):
            XT = xtpool.tile([P, 8, T], f16)
            for cb in range(8):
                nc.sync.dma_start(out=XT[:, cb], in_=xT_v[:, cb])

            with tc.tile_pool(name="xqt", bufs=1) as xqtpool:
                XQT = xqtpool.tile([P, 8, QB * P], f16)
                for cb in range(8):
                    nc.sync.dma_start(out=XQT[:, cb], in_=xqT_v[:, cb])
                # Q^T projection: j-blocks 0..7
                for pb in range(PAIRS):
                    wt = wpool.tile([P, 8, P], f16, tag="w")
                    nc.sync.dma_start(out=wt, in_=wqkv_d[pb])
                    for t4 in range(2):
                        ps = pspool.tile([P, 512], f32, tag="proj")
                        for cb in range(8):
                            nc.tensor.matmul(
                                ps,
                                lhsT=wt[:, cb],
                                rhs=XQT[:, cb, 512 * t4 : 512 * (t4 + 1)],
                                start=(cb == 0),
                                stop=(cb == 7),
                            )
                        nc.scalar.activation(
                            QT[:, pb, 512 * t4 : 512 * (t4 + 1)],
                            ps,
                            AF.Identity,
                            bias=bqs[:, pb : pb + 1],
                        )

            # K^T (j-blocks 8..15) and V^T (j-blocks 16..23) over full T
            for kind, dst in ((1, KT), (2, VT)):
                for pb in range(PAIRS):
                    jb = 8 * kind + pb
                    wt = wpool.tile([P, 8, P], f16, tag="w")
                    nc.sync.dma_start(out=wt, in_=wqkv_d[jb])
                    for t4 in range(4):
                        ps = pspool.tile([P, 512], f32, tag="proj")
                        for cb in range(8):
                            nc.tensor.matmul(
                                ps,
                                lhsT=wt[:, cb],
                                rhs=XT[:, cb, 512 * t4 : 512 * (t4 + 1)],
                                start=(cb == 0),
                                stop=(cb == 7),
                            )
                        nc.scalar.activation(
                            dst[:, pb, 512 * t4 : 512 * (t4 + 1)],
                            ps,
                            AF.Identity,
                            bias=bqs[:, jb : jb + 1],
                        )

        if "kt" in dbg_d:
            nc.sync.dma_start(out=dbg_d["kt"], in_=KT)
            nc.sync.dma_start(out=dbg_d["qt"], in_=QT)

        # V natural layout with ones column: VN[:, pb, kb, h2] = [V_h | 1]
        vnpool = ctx.enter_context(tc.tile_pool(name="vn", bufs=1))
        VN = vnpool.tile([P, PAIRS, KB, 2, 65], f16)
        if dbg_stage != "proj":
            nc.vector.memset(VN[:, :, :, :, 64:65], 1.0)
            with tc.tile_pool(name="pstr", bufs=3, space="PSUM") as pstr:
                for pb in range(PAIRS):
                    for kg in range(4):  # groups of 4 key blocks
                        pt = pstr.tile([P, 512], f16, tag="tr")
                        for j in range(4):
                            kb = kg * 4 + j
                            # full 128-row transpose: out cols = [h0 d | h1 d]
                            nc.tensor.transpose(
                                pt[:, j * 128 : (j + 1) * 128],
                                VT[:, pb, kb * P : (kb + 1) * P],
                                i128,
                            )
                        nc.vector.tensor_copy(
                            out=VN[:, pb, kg * 4 : (kg + 1) * 4, :, 0:64],
                            in_=pt.rearrange("p (a b c) -> p a b c", b=2, c=64),
                        )
            if "vn" in dbg_d:
                nc.sync.dma_start(out=dbg_d["vn"], in_=VN)

        # attention
        attn_pairs = range(PAIRS) if dbg_stage in ("", "attn", "full") else []
        with (
            tc.tile_pool(name="pts", bufs=3) as ptpool,
            tc.tile_pool(name="sm", bufs=3) as smpool,
            tc.tile_pool(name="psS", bufs=2, space="PSUM") as psSpool,
            tc.tile_pool(name="psY", bufs=2, space="PSUM") as psYpool,
        ):
            for pb in attn_pairs:
                for i in range(QB):
                    L = 2 * i + 2
                    psY = [
                        psYpool.tile([P, 65], f32, tag=f"psY{h}", name=f"psY{h}")
                        for h in range(2)
                    ]
                    nch = (L + 3) // 4
                    for ch in range(nch):
                        kbs = list(range(ch * 4, min(ch * 4 + 4, L)))
                        width = len(kbs) * P
                        psS = [
                            psSpool.tile(
                                [P, 512], f32, tag=f"psS{h}", name=f"psS{h}"
                            )[:, :width]
                            for h in range(2)
                        ]
                        pt = [
                            ptpool.tile(
                                [P, 512], f16, tag=f"pt{h}", name=f"pt{h}"
                            )[:, :width]
                            for h in range(2)
                        ]
                        for j, kb in enumerate(kbs):
                            for h in range(2):
                                nc.tensor.matmul(
                                    psS[h][:, j * P : (j + 1) * P],
                                    lhsT=KT[
                                        64 * h : 64 * h + 64, pb, kb * P : (kb + 1) * P
                                    ],
                                    rhs=QT[
                                        64 * h : 64 * h + 64, pb, i * P : (i + 1) * P
                                    ],
                                    start=True,
                                    stop=True,
                                )
                        for h in range(2):
                            nc.scalar.activation(pt[h], psS[h], AF.Exp, scale=0.125)
                        if ch == (2 * i) // 4:
                            off = (2 * i - ch * 4) * P
                            for h in range(2):
                                nc.vector.tensor_mul(
                                    out=pt[h][:, off : off + 256].rearrange(
                                        "p (m q) -> p m q", m=2
                                    ),
                                    in0=pt[h][:, off : off + 256].rearrange(
                                        "p (m q) -> p m q", m=2
                                    ),
                                    in1=msk,
                                )
                        for j, kb in enumerate(kbs):
                            for h in range(2):
                                nc.tensor.matmul(
                                    psY[h],
                                    lhsT=pt[h][:, j * P : (j + 1) * P],
                                    rhs=VN[:, pb, kb, h],
                                    start=(kb == 0),
                                    stop=(kb == L - 1),
                                )
                    pyt = psSpool.tile([P, 512], f16, tag="psS0", name="pyt")[:, :P]
                    yn = smpool.tile([P, P], f16, tag="yn")
                    for h in range(2):
                        r = smpool.tile([P, 1], f32, tag=f"r{h}")
                        nc.vector.reciprocal(r, psY[h][:, 64:65])
                        nc.vector.tensor_scalar_mul(
                            yn[:, 64 * h : 64 * h + 64], psY[h][:, 0:64], r
                        )
                    nc.tensor.transpose(pyt, yn, i128)
                    nc.vector.tensor_copy(
                        out=YT[:, pb, i * P : (i + 1) * P], in_=pyt
                    )

        if "yt" in dbg_d:
            nc.sync.dma_start(out=dbg_d["yt"], in_=YT)
        if dbg_stage:
            # debug stages: make sure yr is written so outputs exist
            nc.vector.memset(YT[:1, 0, :1], 0.0)
        out_blocks = range(QB) if dbg_stage in ("", "attn", "full") else []
        # output projection + bias
        with (
            tc.tile_pool(name="z", bufs=2) as zpool,
            tc.tile_pool(name="psZ", bufs=2, space="PSUM") as psZpool,
        ):
            for i in out_blocks:
                zt = zpool.tile([P, C], f32, tag="z")
                for nc2 in range(2):
                    ps = psZpool.tile([P, 512], f32, tag="z")
                    for cb in range(8):
                        nc.tensor.matmul(
                            ps,
                            lhsT=YT[:, cb, i * P : (i + 1) * P],
                            rhs=WO[:, cb, 512 * nc2 : 512 * (nc2 + 1)],
                            start=(cb == 0),
                            stop=False,
                        )
                    nc.tensor.matmul(
                        ps,
                        lhsT=on1,
                        rhs=bo[:, 512 * nc2 : 512 * (nc2 + 1)],
                        start=False,
                        stop=True,
                    )
                    nc.scalar.copy(zt[:, 512 * nc2 : 512 * (nc2 + 1)], ps)
                nc.sync.dma_start(out=yr_d[:, i], in_=zt)

    split_sync_waits(nc)
    return nc


def _host_inputs(x, w_qkv, b_qkv, w_out, b_out):
    x = np.asarray(x, dtype=np.float32)
    w_qkv = np.asarray(w_qkv, dtype=np.float32)
    b_qkv = np.asarray(b_qkv, dtype=np.float32)
    w_out = np.asarray(w_out, dtype=np.float32)
    b_out = np.asarray(b_out, dtype=np.float32)

    wqkv_r = np.ascontiguousarray(
        w_qkv.reshape(8, P, 24, P).transpose(2, 1, 0, 3)
    ).astype(np.float16)
    wout_r = np.ascontiguousarray(
        w_out.reshape(8, P, C).transpose(1, 0, 2)
    ).astype(np.float16)
    bqkv_r = np.ascontiguousarray(b_qkv.reshape(24, P).T)
    bout_r = b_out.reshape(1, C).astype(np.float16)
    tri = np.triu(np.ones((P, P), dtype=np.float16))  # [k, q]: k <= q
    zer = np.zeros((P, P), dtype=np.float16)
    one = np.ones((P, P), dtype=np.float16)
    id64 = np.zeros((P, 64), dtype=np.float16)
    id64[np.arange(P), np.arange(P) % 64] = 1
    id128 = np.eye(P, dtype=np.float16)
    ones1 = np.ones((1, P), dtype=np.float16)

    in_maps = []
    for c in range(N_CORES):
        b, par = c // 2, c % 2
        xb = x[b]
        xT = np.ascontiguousarray(xb.T).astype(np.float16)
        qg = [2 * i + par for i in range(QB)]
        xq = np.concatenate([xb[g * P : (g + 1) * P] for g in qg], axis=0)
        xqT = np.ascontiguousarray(xq.T).astype(np.float16)
        m = np.stack([tri, zer] if par == 0 else [one, tri])  # [2, k, q]
        mask = np.ascontiguousarray(m.transpose(1, 0, 2))  # [k, 2, q]
        in_maps.append(
            {
                "xt": xT,
                "xqt": xqT,
                "wqkv": wqkv_r,
                "wout": wout_r,
                "bqkv": bqkv_r,
                "bout": bout_r,
                "mask": mask,
                "id64": id64,
                "id128": id128,
                "ones1": ones1,
            }
        )
    return in_maps


def kernel(x, w_qkv, b_qkv, w_out, b_out, trace=False):
    global _COMPILED, LAST_EXEC_NS
    from concourse import bass_utils

    if _COMPILED is None:
        _COMPILED = _build()
    nc = _COMPILED

    in_maps = _host_inputs(x, w_qkv, b_qkv, w_out, b_out)
    res = bass_utils.run_bass_kernel_spmd(
        nc, in_maps, core_ids=list(range(N_CORES)), trace=trace
    )
    LAST_EXEC_NS = res.exec_time_ns

    y = np.empty((B, T, C), dtype=np.float32)
    for c in range(N_CORES):
        b, par = c // 2, c % 2
        yl = res.results[c]["yr"].transpose(1, 0, 2)  # [QB, P, C]
        for i in range(QB):
            g = 2 * i + par
            y[b, g * P : (g + 1) * P] = yl[i]
    return y



# revision 3
# speedup vs baseline: 133.5221x; 133.5221x over previous
"""Causal self-attention Trainium2 kernel (B=4, T=2048, C=1024, H=16, D=64).

Sharding: 8 cores = 4 batches x 2 causally-balanced query shards.
Core c handles batch b=c//2 and the 8 interleaved query blocks
g = 2*i + (c%2), i in 0..7 (block = 128 rows).  Every core computes full-
sequence K/V for its batch plus Q for its own query rows, runs all 16 heads
of attention for those rows, and the full output projection for them, so
per-core outputs are disjoint row-slices of y: no collectives, no host math.

Device-side dataflow (all matmuls fp16 in / fp32 PSUM accumulate):
  qkv^T = w^T @ x^T        (x^T supplied by host; K^T/Q^T land in [d, t]
                            layout with head pairs stacked on partitions)
  S^T   = K @ Q^T          (two heads row-tiled on the PE array, contract=64)
  P^T   = exp(0.125*S^T)   (ACT, psum->sbuf, fp16 out) then 0/1 mask mult
  [Y|s] = P^T.T @ [V|1]    (ones-augmented V gives softmax sums in col 64)
  Yn    = Y * (1/s)        (per-partition scalar on DVE)
  Y^T   via PE transpose   (feeds out-proj as lhsT)
  Z     = Y^T.T @ w_out + ones x b_out  (bias as a K=1 matmul)
"""

import math
import numpy as np

B, T, C = 4, 2048, 1024
H, D = 16, 64
N_CORES = 8
P = 128
QB = 8  # local query blocks per core (of 128 rows)
KB = 16  # key blocks per sequence
PAIRS = 8  # head pairs

_COMPILED = None
LAST_EXEC_NS = None


def _get_mybir():
    import concourse.mybir as mybir
    return mybir


def split_sync_waits(nc):
    mybir = _get_mybir()
    # This walrus build rejects instructions carrying more than one sync
    # wait (or update).  Split the extras onto NOP carriers: waits go on
    # NOPs inserted before the instruction (same engine, so they gate it),
    # updates onto NOPs after it (fire once it has completed).
    uid = [0]

    def carrier(engine, wait=None, update=None):
        uid[0] += 1
        n = mybir.InstNoOp(
            name=f"I-syncsplit-{uid[0]}",
            opcode="NoOp",
            ins=[],
            outs=[],
            sync_info=mybir.SyncInfo(
                on_wait=[wait] if wait else [],
                on_update=[update] if update else [],
            ),
        )
        n.engine = engine
        return n

    import os

    debug = os.environ.get("SYNC_SPLIT_DEBUG")
    for f in nc.m.functions:
        for blk in f.blocks:
            out = []
            changed = False
            for inst in blk.instructions:
                si = inst.sync_info
                if si is None or (
                    len(si.on_wait) <= 1 and len(si.on_update) <= 1
                ):
                    out.append(inst)
                    continue
                if debug:
                    print(
                        f"SPLIT {inst.opcode} {inst.name} eng={inst.engine} "
                        f"waits={len(si.on_wait)} upds={len(si.on_update)}"
                    )
                changed = True
                waits = list(si.on_wait)
                updates = list(si.on_update)
                for w in waits[1:]:
                    out.append(carrier(inst.engine, wait=w))
                inst.sync_info = mybir.SyncInfo(
                    on_wait=waits[:1], on_update=updates[:1]
                )
                out.append(inst)
                for u in updates[1:]:
                    out.append(carrier(inst.engine, update=u))
            if changed:
                blk.instructions = out

import os

dbg_stage = os.environ.get("KERNEL_DEBUG_STAGE", "")




def _build(reps=1):
    import concourse.bass as bass
    import concourse.tile as tile
    import concourse.mybir as mybir
    from concourse.vector_clock import ScopedClock
    from contextlib import ExitStack, nullcontext

    f32 = mybir.dt.float32
    f16 = mybir.dt.float16
    AF = mybir.ActivationFunctionType

    nc = bass.Bass(
        "TRN2", target_bir_lowering=False, debug=False, num_devices=N_CORES
    )

    xT_d = nc.dram_tensor("xt", [C, T], f16, kind="ExternalInput").ap()
    xqT_d = nc.dram_tensor("xqt", [C, QB * P], f16, kind="ExternalInput").ap()
    wqkv_d = nc.dram_tensor("wqkv", [24, P, 8, P], f16, kind="ExternalInput").ap()
    wout_d = nc.dram_tensor("wout", [P, 8, C], f16, kind="ExternalInput").ap()
    bqkv_d = nc.dram_tensor("bqkv", [P, 24], f32, kind="ExternalInput").ap()
    bout_d = nc.dram_tensor("bout", [1, C], f16, kind="ExternalInput").ap()
    mask_d = nc.dram_tensor("mask", [P, 2, P], f16, kind="ExternalInput").ap()
    id64_d = nc.dram_tensor("id64", [P, 64], f16, kind="ExternalInput").ap()
    id128_d = nc.dram_tensor("id128", [P, P], f16, kind="ExternalInput").ap()
    ones_d = nc.dram_tensor("ones1", [1, P], f16, kind="ExternalInput").ap()
    yr_d = nc.dram_tensor("yr", [P, QB, C], f32, kind="ExternalOutput").ap()
    dbg_d = {}
    if dbg_stage in ("proj", "vn", "attn"):
        dbg_d["kt"] = nc.dram_tensor(
            "kt", [P, PAIRS, T], f16, kind="ExternalOutput"
        ).ap()
        dbg_d["qt"] = nc.dram_tensor(
            "qt", [P, PAIRS, QB * P], f16, kind="ExternalOutput"
        ).ap()
    if dbg_stage in ("vn", "attn"):
        dbg_d["vn"] = nc.dram_tensor(
            "vn", [P, PAIRS, KB, 2, 65], f16, kind="ExternalOutput"
        ).ap()
    if dbg_stage == "attn":
        dbg_d["yt"] = nc.dram_tensor(
            "yt", [P, 8, QB * P], f16, kind="ExternalOutput"
        ).ap()

    with tile.TileContext(nc) as tc, (
        tc.For_i(0, reps) if reps > 1 else nullcontext()
    ), ExitStack() as ctx:
        persist = ctx.enter_context(tc.tile_pool(name="persist", bufs=1))
        KT = persist.tile([P, PAIRS, T], f16)  # K^T + bias, head pairs stacked
        QT = persist.tile([P, PAIRS, QB * P], f16)
        YT = persist.tile([P, 8, QB * P], f16)  # attention out, [c, tq]
        WO = persist.tile([P, 8, C], f16)
        msk = persist.tile([P, 2, P], f16)
        bqs = persist.tile([P, 24], f32)
        i64 = persist.tile([P, 64], f16)
        i128 = persist.tile([P, P], f16)
        on1 = persist.tile([1, P], f16)
        bo = persist.tile([1, C], f16)

        nc.sync.dma_start(out=msk, in_=mask_d)
        nc.sync.dma_start(out=bqs, in_=bqkv_d)
        nc.sync.dma_start(out=i64, in_=id64_d)
        nc.sync.dma_start(out=i128, in_=id128_d)
        nc.sync.dma_start(out=on1, in_=ones_d)
        nc.sync.dma_start(out=bo, in_=bout_d)
        nc.sync.dma_start(out=WO, in_=wout_d)

        wpool = ctx.enter_context(tc.tile_pool(name="w", bufs=3))
        vtpool = ctx.enter_context(tc.tile_pool(name="vt", bufs=1))
        VT = vtpool.tile([P, PAIRS, T], f16)

        xT_v = xT_d.rearrange("(cb p) t -> p cb t", p=P)
        xqT_v = xqT_d.rearrange("(cb p) t -> p cb t", p=P)

        with (
            tc.tile_pool(name="xt", bufs=1) as xtpool,
            tc.tile_pool(name="psproj", bufs=3, space="PSUM") as pspool,
        ):
            XT = xtpool.tile([P, 8, T], f16)
            for cb in range(8):
                nc.sync.dma_start(out=XT[:, cb], in_=xT_v[:, cb])

            with tc.tile_pool(name="xqt", bufs=1) as xqtpool:
                XQT = xqtpool.tile([P, 8, QB * P], f16)
                for cb in range(8):
                    nc.sync.dma_start(out=XQT[:, cb], in_=xqT_v[:, cb])
                # Q^T projection: j-blocks 0..7
                for pb in range(PAIRS):
                    wt = wpool.tile([P, 8, P], f16, tag="w")
                    nc.sync.dma_start(out=wt, in_=wqkv_d[pb])
                    for t4 in range(2):
                        ps = pspool.tile([P, 512], f32, tag="proj")
                        for cb in range(8):
                            nc.tensor.matmul(
                                ps,
                                lhsT=wt[:, cb],
                                rhs=XQT[:, cb, 512 * t4 : 512 * (t4 + 1)],
                                start=(cb == 0),
                                stop=(cb == 7),
                            )
                        nc.scalar.activation(
                            QT[:, pb, 512 * t4 : 512 * (t4 + 1)],
                            ps,
                            AF.Identity,
                            bias=bqs[:, pb : pb + 1],
                        )

            # K^T (j-blocks 8..15) and V^T (j-blocks 16..23) over full T
            for kind, dst in ((1, KT), (2, VT)):
                for pb in range(PAIRS):
                    jb = 8 * kind + pb
                    wt = wpool.tile([P, 8, P], f16, tag="w")
                    nc.sync.dma_start(out=wt, in_=wqkv_d[jb])
                    for t4 in range(4):
                        ps = pspool.tile([P, 512], f32, tag="proj")
                        for cb in range(8):
                            nc.tensor.matmul(
                                ps,
                                lhsT=wt[:, cb],
                                rhs=XT[:, cb, 512 * t4 : 512 * (t4 + 1)],
                                start=(cb == 0),
                                stop=(cb == 7),
                            )
                        nc.scalar.activation(
                            dst[:, pb, 512 * t4 : 512 * (t4 + 1)],
                            ps,
                            AF.Identity,
                            bias=bqs[:, jb : jb + 1],
                        )

        if "kt" in dbg_d:
            nc.sync.dma_start(out=dbg_d["kt"], in_=KT)
            nc.sync.dma_start(out=dbg_d["qt"], in_=QT)

        # V natural layout with ones column: VN[:, pb, kb, h2] = [V_h | 1]
        vnpool = ctx.enter_context(tc.tile_pool(name="vn", bufs=1))
        VN = vnpool.tile([P, PAIRS, KB, 2, 65], f16)
        if dbg_stage != "proj":
            nc.vector.memset(VN[:, :, :, :, 64:65], 1.0)
            with tc.tile_pool(name="pstr", bufs=3, space="PSUM") as pstr:
                for pb in range(PAIRS):
                    for kg in range(4):  # groups of 4 key blocks
                        pt = pstr.tile([P, 512], f16, tag="tr")
                        for j in range(4):
                            kb = kg * 4 + j
                            # full 128-row transpose: out cols = [h0 d | h1 d]
                            nc.tensor.transpose(
                                pt[:, j * 128 : (j + 1) * 128],
                                VT[:, pb, kb * P : (kb + 1) * P],
                                i128,
                            )
                        nc.vector.tensor_copy(
                            out=VN[:, pb, kg * 4 : (kg + 1) * 4, :, 0:64],
                            in_=pt.rearrange("p (a b c) -> p a b c", b=2, c=64),
                        )
            if "vn" in dbg_d:
                nc.sync.dma_start(out=dbg_d["vn"], in_=VN)

        # attention
        attn_pairs = range(PAIRS) if dbg_stage in ("", "attn", "full") else []
        with (
            tc.tile_pool(name="pts", bufs=3) as ptpool,
            tc.tile_pool(name="sm", bufs=3) as smpool,
            tc.tile_pool(name="psS", bufs=2, space="PSUM") as psSpool,
            tc.tile_pool(name="psY", bufs=2, space="PSUM") as psYpool,
        ):
            for pb in attn_pairs:
                for i in range(QB):
                    L = 2 * i + 2
                    psY = [
                        psYpool.tile([P, 65], f32, tag=f"psY{h}", name=f"psY{h}")
                        for h in range(2)
                    ]
                    nch = (L + 3) // 4
                    for ch in range(nch):
                        kbs = list(range(ch * 4, min(ch * 4 + 4, L)))
                        width = len(kbs) * P
                        psS = [
                            psSpool.tile(
                                [P, 512], f32, tag=f"psS{h}", name=f"psS{h}"
                            )[:, :width]
                            for h in range(2)
                        ]
                        pt = [
                            ptpool.tile(
                                [P, 512], f16, tag=f"pt{h}", name=f"pt{h}"
                            )[:, :width]
                            for h in range(2)
                        ]
                        for j, kb in enumerate(kbs):
                            for h in range(2):
                                nc.tensor.matmul(
                                    psS[h][:, j * P : (j + 1) * P],
                                    lhsT=KT[
                                        64 * h : 64 * h + 64, pb, kb * P : (kb + 1) * P
                                    ],
                                    rhs=QT[
                                        64 * h : 64 * h + 64, pb, i * P : (i + 1) * P
                                    ],
                                    start=True,
                                    stop=True,
                                )
                        for h in range(2):
                            nc.scalar.activation(pt[h], psS[h], AF.Exp, scale=0.125)
                        if ch == (2 * i) // 4:
                            off = (2 * i - ch * 4) * P
                            for h in range(2):
                                nc.vector.tensor_mul(
                                    out=pt[h][:, off : off + 256].rearrange(
                                        "p (m q) -> p m q", m=2
                                    ),
                                    in0=pt[h][:, off : off + 256].rearrange(
                                        "p (m q) -> p m q", m=2
                                    ),
                                    in1=msk,
                                )
                        for j, kb in enumerate(kbs):
                            for h in range(2):
                                nc.tensor.matmul(
                                    psY[h],
                                    lhsT=pt[h][:, j * P : (j + 1) * P],
                                    rhs=VN[:, pb, kb, h],
                                    start=(kb == 0),
                                    stop=(kb == L - 1),
                                )
                    pyt = psSpool.tile([P, 512], f16, tag="psS0", name="pyt")[:, :P]
                    yn = smpool.tile([P, P], f16, tag="yn")
                    for h in range(2):
                        r = smpool.tile([P, 1], f32, tag=f"r{h}")
                        nc.vector.reciprocal(r, psY[h][:, 64:65])
                        nc.vector.tensor_scalar_mul(
                            yn[:, 64 * h : 64 * h + 64], psY[h][:, 0:64], r
                        )
                    nc.tensor.transpose(pyt, yn, i128)
                    nc.vector.tensor_copy(
                        out=YT[:, pb, i * P : (i + 1) * P], in_=pyt
                    )

        if "yt" in dbg_d:
            nc.sync.dma_start(out=dbg_d["yt"], in_=YT)
        if dbg_stage:
            # debug stages: make sure yr is written so outputs exist
            nc.vector.memset(YT[:1, 0, :1], 0.0)
        out_blocks = range(QB) if dbg_stage in ("", "attn", "full") else []
        # output projection + bias
        with (
            tc.tile_pool(name="z", bufs=2) as zpool,
            tc.tile_pool(name="psZ", bufs=2, space="PSUM") as psZpool,
        ):
            for i in out_blocks:
                zt = zpool.tile([P, C], f32, tag="z")
                for nc2 in range(2):
                    ps = psZpool.tile([P, 512], f32, tag="z")
                    for cb in range(8):
                        nc.tensor.matmul(
                            ps,
                            lhsT=YT[:, cb, i * P : (i + 1) * P],
                            rhs=WO[:, cb, 512 * nc2 : 512 * (nc2 + 1)],
                            start=(cb == 0),
                            stop=False,
                        )
                    nc.tensor.matmul(
                        ps,
                        lhsT=on1,
                        rhs=bo[:, 512 * nc2 : 512 * (nc2 + 1)],
                        start=False,
                        stop=True,
                    )
                    nc.scalar.copy(zt[:, 512 * nc2 : 512 * (nc2 + 1)], ps)
                nc.sync.dma_start(out=yr_d[:, i], in_=zt)

    split_sync_waits(nc)
    return nc


def _host_inputs(x, w_qkv, b_qkv, w_out, b_out):
    x = np.asarray(x, dtype=np.float32)
    w_qkv = np.asarray(w_qkv, dtype=np.float32)
    b_qkv = np.asarray(b_qkv, dtype=np.float32)
    w_out = np.asarray(w_out, dtype=np.float32)
    b_out = np.asarray(b_out, dtype=np.float32)

    wqkv_r = np.ascontiguousarray(
        w_qkv.reshape(8, P, 24, P).transpose(2, 1, 0, 3)
    ).astype(np.float16)
    wout_r = np.ascontiguousarray(
        w_out.reshape(8, P, C).transpose(1, 0, 2)
    ).astype(np.float16)
    bqkv_r = np.ascontiguousarray(b_qkv.reshape(24, P).T)
    bout_r = b_out.reshape(1, C).astype(np.float16)
    tri = np.triu(np.ones((P, P), dtype=np.float16))  # [k, q]: k <= q
    zer = np.zeros((P, P), dtype=np.float16)
    one = np.ones((P, P), dtype=np.float16)
    id64 = np.zeros((P, 64), dtype=np.float16)
    id64[np.arange(P), np.arange(P) % 64] = 1
    id128 = np.eye(P, dtype=np.float16)
    ones1 = np.ones((1, P), dtype=np.float16)

    in_maps = []
    for c in range(N_CORES):
        b, par = c // 2, c % 2
        xb = x[b]
        xT = np.ascontiguousarray(xb.T).astype(np.float16)
        qg = [2 * i + par for i in range(QB)]
        xq = np.concatenate([xb[g * P : (g + 1) * P] for g in qg], axis=0)
        xqT = np.ascontiguousarray(xq.T).astype(np.float16)
        m = np.stack([tri, zer] if par == 0 else [one, tri])  # [2, k, q]
        mask = np.ascontiguousarray(m.transpose(1, 0, 2))  # [k, 2, q]
        in_maps.append(
            {
                "xt": xT,
                "xqt": xqT,
                "wqkv": wqkv_r,
                "wout": wout_r,
                "bqkv": bqkv_r,
                "bout": bout_r,
                "mask": mask,
                "id64": id64,
                "id128": id128,
                "ones1": ones1,
            }
        )
    return in_maps


def kernel(x, w_qkv, b_qkv, w_out, b_out, trace=False):
    global _COMPILED, LAST_EXEC_NS
    from concourse import bass_utils

    if _COMPILED is None:
        _COMPILED = _build()
    nc = _COMPILED

    in_maps = _host_inputs(x, w_qkv, b_qkv, w_out, b_out)
    res = bass_utils.run_bass_kernel_spmd(
        nc, in_maps, core_ids=list(range(N_CORES)), trace=trace
    )
    LAST_EXEC_NS = res.exec_time_ns

    y = np.empty((B, T, C), dtype=np.float32)
    for c in range(N_CORES):
        b, par = c // 2, c % 2
        yl = res.results[c]["yr"].transpose(1, 0, 2)  # [QB, P, C]
        for i in range(QB):
            g = 2 * i + par
            y[b, g * P : (g + 1) * P] = yl[i]
    return y



# revision 17
# speedup vs baseline: 142.0782x; 1.0641x over previous
"""Causal self-attention Trainium2 kernel (B=4, T=2048, C=1024, H=16, D=64).

Sharding: 8 cores = 4 batches x 2 causally-balanced query shards.
Core c handles batch b=c//2 and the 8 interleaved query blocks
g = 2*i + (c%2), i in 0..7 (block = 128 rows).  Every core computes full-
sequence K/V for its batch plus Q for its own query rows, runs all 16 heads
of attention for those rows, and the full output projection for them, so
per-core outputs are disjoint row-slices of y: no collectives, no host math.

Device-side dataflow (matmuls fp16 in / fp32 PSUM):
  K^T, Q^T = w^T @ x^T        (transposed projection, [d, t] layout,
                               head pairs stacked on partitions; bias on DVE)
  V        = x @ w_v          (natural [t, d] layout via XT as lhsT; bias on
                               DVE from a GPSIMD-broadcast row; ones col in
                               VN col 64 gives softmax sums during AV)
  S^T      = K @ Q^T          (kb-outer, queries streamed wide: N up to 512;
                               the two heads of a pair run CONCURRENTLY via
                               64-partition row tiling of the PE array)
  P^T      = exp(0.125*S^T)   (one ACT op per (kb): [128, 2, w] psum->sbuf)
  mask     on DVE             (always first 128 query cols; data-dependent
                               per core parity: tri/zero vs ones/tri)
  Y^T[d,q] = VN^T @ P^T       (transposed AV: lhsT=VN [k,65], rhs=P [k,q],
                               accumulated over kb in PSUM; row 64 = sums)
  Yn^T     = Y^T * (1/s)      (DVE recip + GPSIMD partition_broadcast + DVE
                               mult straight into YT, the out-proj lhsT)
  Z        = YT.T @ w_out + ones x b_out  (bias as a K=1 matmul; copies DVE)

Emission interleaves proj(pb+1) with attn(pb) so PE projection work fills
the ACT-bound attention window.
"""

import math
import os
import numpy as np

B, T, C = 4, 2048, 1024
H, D = 16, 64
N_CORES = 8
P = 128
QB = 8  # local query blocks per core (of 128 rows)
KB = 16  # key blocks per sequence
PAIRS = 8  # head pairs

_COMPILED = None
LAST_EXEC_NS = None

dbg_stage = os.environ.get("KERNEL_DEBUG_STAGE", "")


def _get_mybir():
    import concourse.mybir as mybir
    return mybir


def split_sync_waits(nc):
    mybir = _get_mybir()
    # This walrus build rejects instructions carrying more than one sync
    # wait (or update).  Split the extras onto NOP carriers: waits go on
    # NOPs inserted before the instruction (same engine, so they gate it),
    # updates onto NOPs after it (fire once it has completed).
    uid = [0]

    def carrier(engine, wait=None, update=None):
        uid[0] += 1
        n = mybir.InstNoOp(
            name=f"I-syncsplit-{uid[0]}",
            opcode="NoOp",
            ins=[],
            outs=[],
            sync_info=mybir.SyncInfo(
                on_wait=[wait] if wait else [],
                on_update=[update] if update else [],
            ),
        )
        n.engine = engine
        return n

    debug = os.environ.get("SYNC_SPLIT_DEBUG")
    for f in nc.m.functions:
        for blk in f.blocks:
            out = []
            changed = False
            for inst in blk.instructions:
                si = inst.sync_info
                if si is None or (
                    len(si.on_wait) <= 1 and len(si.on_update) <= 1
                ):
                    out.append(inst)
                    continue
                if debug:
                    print(
                        f"SPLIT {inst.opcode} {inst.name} eng={inst.engine} "
                        f"waits={len(si.on_wait)} upds={len(si.on_update)}"
                    )
                changed = True
                waits = list(si.on_wait)
                updates = list(si.on_update)
                for w in waits[1:]:
                    out.append(carrier(inst.engine, wait=w))
                inst.sync_info = mybir.SyncInfo(
                    on_wait=waits[:1], on_update=updates[:1]
                )
                out.append(inst)
                for u in updates[1:]:
                    out.append(carrier(inst.engine, update=u))
            if changed:
                blk.instructions = out


def _build(reps=1):
    import concourse.bass as bass
    import concourse.tile as tile
    import concourse.mybir as mybir
    from contextlib import ExitStack, nullcontext

    f32 = mybir.dt.float32
    f16 = mybir.dt.float16
    AF = mybir.ActivationFunctionType

    nc = bass.Bass(
        "TRN2", target_bir_lowering=False, debug=False, num_devices=N_CORES
    )

    xT_d = nc.dram_tensor("xt", [C, T], f16, kind="ExternalInput").ap()
    xqT_d = nc.dram_tensor("xqt", [C, QB * P], f16, kind="ExternalInput").ap()
    wqkv_d = nc.dram_tensor("wqkv", [16, P, 8, P], f16, kind="ExternalInput").ap()
    wv_d = nc.dram_tensor("wv", [P, 8, C], f16, kind="ExternalInput").ap()
    wout_d = nc.dram_tensor("wout", [P, 8, C], f16, kind="ExternalInput").ap()
    bqkv_d = nc.dram_tensor("bqkv", [P, 24], f32, kind="ExternalInput").ap()
    bvb_d = nc.dram_tensor("bvb", [P, C], f16, kind="ExternalInput").ap()
    bout_d = nc.dram_tensor("bout", [1, C], f16, kind="ExternalInput").ap()
    mskD_d = nc.dram_tensor("mskD", [P, 2, 2, P], f16, kind="ExternalInput").ap()
    ones_d = nc.dram_tensor("ones1", [1, P], f16, kind="ExternalInput").ap()
    yr_d = nc.dram_tensor("yr", [P, QB, C], f32, kind="ExternalOutput").ap()
    dbg_d = {}
    if dbg_stage in ("proj", "vn", "attn"):
        dbg_d["kt"] = nc.dram_tensor(
            "kt", [P, PAIRS, T], f16, kind="ExternalOutput"
        ).ap()
        dbg_d["qt"] = nc.dram_tensor(
            "qt", [P, PAIRS, QB * P], f16, kind="ExternalOutput"
        ).ap()
    if dbg_stage in ("vn", "attn"):
        dbg_d["vn"] = nc.dram_tensor(
            "vn", [P, PAIRS, KB, 2, 65], f16, kind="ExternalOutput"
        ).ap()
    if dbg_stage == "attn":
        dbg_d["yt"] = nc.dram_tensor(
            "yt", [P, 8, QB * P], f16, kind="ExternalOutput"
        ).ap()

    with tile.TileContext(nc) as tc, (
        tc.For_i(0, reps) if reps > 1 else nullcontext()
    ), ExitStack() as ctx:
        persist = ctx.enter_context(tc.tile_pool(name="persist", bufs=1))
        KT = persist.tile([P, PAIRS, T], f16)  # K^T + bias, head pairs stacked
        QT = persist.tile([P, PAIRS, QB * P], f16)
        YT = persist.tile([P, 8, QB * P], f16)  # normalized attn out, [c, tq]
        VN = persist.tile([P, PAIRS, KB, 2, 65], f16)  # [V_h | 1], natural
        WO = persist.tile([P, 8, C], f16)
        WV = persist.tile([P, 8, C], f16)
        mskD = persist.tile([P, 2, 2, P], f16)
        bqs = persist.tile([P, 24], f32)
        bvb = persist.tile([P, C], f16)
        on1 = persist.tile([1, P], f16)
        bo = persist.tile([1, C], f16)

        nc.sync.dma_start(out=mskD, in_=mskD_d)
        nc.sync.dma_start(out=bqs, in_=bqkv_d)
        nc.sync.dma_start(out=bvb, in_=bvb_d)
        nc.sync.dma_start(out=on1, in_=ones_d)
        nc.sync.dma_start(out=bo, in_=bout_d)
        nc.sync.dma_start(out=WO, in_=wout_d)
        nc.sync.dma_start(out=WV, in_=wv_d)

        nc.vector.memset(VN[:, :, :, :, 64:65], 1.0)

        # attention pools (outlive the proj scope)
        ptpool = ctx.enter_context(tc.tile_pool(name="pt", bufs=2))
        rpool = ctx.enter_context(tc.tile_pool(name="r", bufs=2))
        psSpool = ctx.enter_context(
            tc.tile_pool(name="psS", bufs=2, space="PSUM")
        )
        psYpool = ctx.enter_context(
            tc.tile_pool(name="psY", bufs=1, space="PSUM")
        )
        wpool = ctx.enter_context(tc.tile_pool(name="w", bufs=2))

        def attn(pb):
            for chunk in range(2):
                nkb = 8 * (chunk + 1)
                psY = [
                    psYpool.tile([P, 512], f32, tag=f"psY{h}", name=f"psY{h}")
                    for h in range(2)
                ]
                for kb in range(nkb):
                    i0 = max(kb // 2, 4 * chunk)
                    w = (4 * (chunk + 1) - i0) * P
                    q0 = i0 * P
                    ql = q0 - 512 * chunk
                    psS = psSpool.tile([P, 2, 512], f32, tag="psS", name="psS")
                    for h in range(2):
                        nc.tensor.matmul(
                            psS[:, h, :w],
                            lhsT=KT[64 * h : 64 * h + 64, pb, kb * P : (kb + 1) * P],
                            rhs=QT[64 * h : 64 * h + 64, pb, q0 : q0 + w],
                            start=True,
                            stop=True,
                        )
                    pt = ptpool.tile([P, 2, 512], f16, tag="pt")
                    nc.scalar.activation(
                        pt[:, :, :w], psS[:, :, :w], AF.Exp, scale=0.125
                    )
                    if chunk == 0 or kb >= 8:
                        nc.vector.tensor_mul(
                            out=pt[:, :, 0:P],
                            in0=pt[:, :, 0:P],
                            in1=mskD[:, kb % 2],
                        )
                    for h in range(2):
                        nc.tensor.matmul(
                            psY[h][0:65, ql : ql + w],
                            lhsT=VN[:, pb, kb, h],
                            rhs=pt[:, h, :w],
                            start=(kb == 0),
                            stop=(kb == nkb - 1),
                            skip_group_check=True,
                        )
                for h in range(2):
                    r = rpool.tile([32, 512], f16, tag=f"r{h}")
                    rb = rpool.tile([32, 512], f16, tag=f"rb{h}")
                    with nc.allow_low_precision(reason="softmax recip in f16"):
                        nc.vector.reciprocal(r[0:1], psY[h][64:65, :])
                    nc.vector.stream_shuffle(rb, r, [0] * 32)
                    yto = YT[:, pb, chunk * 512 : (chunk + 1) * 512]
                    for pq in range(2):
                        nc.vector.tensor_mul(
                            out=yto[64 * h + 32 * pq : 64 * h + 32 * pq + 32],
                            in0=psY[h][32 * pq : 32 * pq + 32],
                            in1=rb,
                        )

        xT_v = xT_d.rearrange("(cb p) t -> p cb t", p=P)
        xqT_v = xqT_d.rearrange("(cb p) t -> p cb t", p=P)

        with (
            tc.tile_pool(name="xt", bufs=1) as xtpool,
            tc.tile_pool(name="psproj", bufs=2, space="PSUM") as pspool,
        ):
            XT = xtpool.tile([P, 8, T], f16)
            XQT = xtpool.tile([P, 8, QB * P], f16)
            for cb in range(8):
                nc.sync.dma_start(out=XT[:, cb], in_=xT_v[:, cb])
            for cb in range(8):
                nc.sync.dma_start(out=XQT[:, cb], in_=xqT_v[:, cb])

            def proj_kq(pb):
                # K^T: j-block 8+pb over full T; Q^T: j-block pb over own rows
                wt = wpool.tile([P, 8, P], f16, tag="w")
                nc.sync.dma_start(out=wt, in_=wqkv_d[8 + pb])
                for t4 in range(4):
                    ps = pspool.tile([P, 512], f32, tag="proj")
                    for cb in range(8):
                        nc.tensor.matmul(
                            ps,
                            lhsT=wt[:, cb],
                            rhs=XT[:, cb, 512 * t4 : 512 * (t4 + 1)],
                            start=(cb == 0),
                            stop=(cb == 7),
                        )
                    nc.vector.tensor_scalar_add(
                        out=KT[:, pb, 512 * t4 : 512 * (t4 + 1)],
                        in0=ps,
                        scalar1=bqs[:, 8 + pb : 9 + pb],
                    )
                wt = wpool.tile([P, 8, P], f16, tag="w")
                nc.sync.dma_start(out=wt, in_=wqkv_d[pb])
                for t2 in range(2):
                    ps = pspool.tile([P, 512], f32, tag="proj")
                    for cb in range(8):
                        nc.tensor.matmul(
                            ps,
                            lhsT=wt[:, cb],
                            rhs=XQT[:, cb, 512 * t2 : 512 * (t2 + 1)],
                            start=(cb == 0),
                            stop=(cb == 7),
                        )
                    nc.vector.tensor_scalar_add(
                        out=QT[:, pb, 512 * t2 : 512 * (t2 + 1)],
                        in0=ps,
                        scalar1=bqs[:, pb : pb + 1],
                    )

            def proj_v(half):
                # V natural: out [t-block, 512 v-cols] = pairs 4*half..4*half+3
                bslice = bvb[:, 512 * half : 512 * (half + 1)].rearrange(
                    "p (a b c) -> p a b c", b=2, c=64
                )
                for tb in range(KB):
                    ps = pspool.tile([P, 512], f32, tag="proj")
                    for cb in range(8):
                        nc.tensor.matmul(
                            ps,
                            lhsT=XT[:, cb, tb * P : (tb + 1) * P],
                            rhs=WV[:, cb, 512 * half : 512 * (half + 1)],
                            start=(cb == 0),
                            stop=(cb == 7),
                        )
                    nc.vector.tensor_add(
                        out=VN[:, 4 * half : 4 * half + 4, tb, :, 0:64],
                        in0=ps.rearrange("p (a b c) -> p a b c", b=2, c=64),
                        in1=bslice,
                    )

            attn_on = dbg_stage in ("", "attn", "full")
            proj_kq(0)
            proj_v(0)
            for pb in range(PAIRS):
                if pb + 1 < PAIRS:
                    proj_kq(pb + 1)
                if pb == 2:
                    proj_v(1)
                if attn_on:
                    attn(pb)

        if "kt" in dbg_d:
            nc.sync.dma_start(out=dbg_d["kt"], in_=KT)
            nc.sync.dma_start(out=dbg_d["qt"], in_=QT)
        if "vn" in dbg_d:
            nc.sync.dma_start(out=dbg_d["vn"], in_=VN)
        if "yt" in dbg_d:
            nc.sync.dma_start(out=dbg_d["yt"], in_=YT)
        if dbg_stage:
            # debug stages: make sure yr is written so outputs exist
            nc.vector.memset(YT[:1, 0, :1], 0.0)
        out_blocks = range(QB) if dbg_stage in ("", "attn", "full") else []
        # output projection + bias
        with (
            tc.tile_pool(name="z", bufs=2) as zpool,
            tc.tile_pool(name="psZ", bufs=2, space="PSUM") as psZpool,
        ):
            for i in out_blocks:
                zt = zpool.tile([P, C], f32, tag="z")
                for nc2 in range(2):
                    ps = psZpool.tile([P, 512], f32, tag="z")
                    for cb in range(8):
                        nc.tensor.matmul(
                            ps,
                            lhsT=YT[:, cb, i * P : (i + 1) * P],
                            rhs=WO[:, cb, 512 * nc2 : 512 * (nc2 + 1)],
                            start=(cb == 0),
                            stop=False,
                        )
                    nc.tensor.matmul(
                        ps,
                        lhsT=on1,
                        rhs=bo[:, 512 * nc2 : 512 * (nc2 + 1)],
                        start=False,
                        stop=True,
                    )
                    nc.vector.tensor_copy(
                        out=zt[:, 512 * nc2 : 512 * (nc2 + 1)], in_=ps
                    )
                nc.sync.dma_start(out=yr_d[:, i], in_=zt)

    split_sync_waits(nc)
    return nc


def _host_inputs(x, w_qkv, b_qkv, w_out, b_out):
    x = np.asarray(x, dtype=np.float32)
    w_qkv = np.asarray(w_qkv, dtype=np.float32)
    b_qkv = np.asarray(b_qkv, dtype=np.float32)
    w_out = np.asarray(w_out, dtype=np.float32)
    b_out = np.asarray(b_out, dtype=np.float32)

    wqkv_r = np.ascontiguousarray(
        w_qkv.reshape(8, P, 24, P).transpose(2, 1, 0, 3)[0:16]
    ).astype(np.float16)
    wv_r = np.ascontiguousarray(
        w_qkv[:, 2 * C : 3 * C].reshape(8, P, C).transpose(1, 0, 2)
    ).astype(np.float16)
    wout_r = np.ascontiguousarray(
        w_out.reshape(8, P, C).transpose(1, 0, 2)
    ).astype(np.float16)
    bqkv_r = np.ascontiguousarray(b_qkv.reshape(24, P).T)
    bvb_r = np.ascontiguousarray(
        np.broadcast_to(b_qkv[2 * C : 3 * C], (P, C))
    ).astype(np.float16)
    bout_r = b_out.reshape(1, C).astype(np.float16)
    tri = np.triu(np.ones((P, P), dtype=np.float16))  # [k, q]: k <= q
    zer = np.zeros((P, P), dtype=np.float16)
    one = np.ones((P, P), dtype=np.float16)
    ones1 = np.ones((1, P), dtype=np.float16)

    in_maps = []
    for c in range(N_CORES):
        b, par = c // 2, c % 2
        xb = x[b]
        xT = np.ascontiguousarray(xb.T).astype(np.float16)
        qg = [2 * i + par for i in range(QB)]
        xq = np.concatenate([xb[g * P : (g + 1) * P] for g in qg], axis=0)
        xqT = np.ascontiguousarray(xq.T).astype(np.float16)
        m0, m1 = (tri, zer) if par == 0 else (one, tri)  # [even kb, odd kb]
        mskD = np.ascontiguousarray(
            np.stack([np.stack([m0, m0]), np.stack([m1, m1])]).transpose(2, 0, 1, 3)
        )  # [k, kb%2, h, q]
        in_maps.append(
            {
                "xt": xT,
                "xqt": xqT,
                "wqkv": wqkv_r,
                "wv": wv_r,
                "wout": wout_r,
                "bqkv": bqkv_r,
                "bvb": bvb_r,
                "bout": bout_r,
                "mskD": mskD,
                "ones1": ones1,
            }
        )
    return in_maps


def kernel(x, w_qkv, b_qkv, w_out, b_out, trace=False):
    global _COMPILED, LAST_EXEC_NS
    from concourse import bass_utils

    if _COMPILED is None:
        _COMPILED = _build()
    nc = _COMPILED

    in_maps = _host_inputs(x, w_qkv, b_qkv, w_out, b_out)
    res = bass_utils.run_bass_kernel_spmd(
        nc, in_maps, core_ids=list(range(N_CORES)), trace=trace
    )
    LAST_EXEC_NS = res.exec_time_ns

    y = np.empty((B, T, C), dtype=np.float32)
    for c in range(N_CORES):
        b, par = c // 2, c % 2
        yl = res.results[c]["yr"].transpose(1, 0, 2)  # [QB, P, C]
        for i in range(QB):
            g = 2 * i + par
            y[b, g * P : (g + 1) * P] = yl[i]
    return y


# revision 22
# speedup vs baseline: 149.4628x; 1.0520x over previous
"""Causal self-attention Trainium2 kernel (B=4, T=2048, C=1024, H=16, D=64).

Sharding: 8 cores = 4 batches x 2 causally-balanced query shards.
Core c handles batch b=c//2 and the 8 interleaved query blocks
g = 2*i + (c%2), i in 0..7 (block = 128 rows).  Every core computes full-
sequence K/V for its batch plus Q for its own query rows, runs all 16 heads
of attention for those rows, and the full output projection for them, so
per-core outputs are disjoint row-slices of y: no collectives, no host math.

Device-side dataflow (matmuls fp16 in / fp32 PSUM):
  K^T, Q^T = w^T @ x^T        (transposed projection, [d, t] layout,
                               head pairs stacked on partitions; bias on DVE)
  V        = x @ w_v          (natural [t, d] layout via XT as lhsT; bias on
                               DVE from a GPSIMD-broadcast row; ones col in
                               VN col 64 gives softmax sums during AV)
  S^T      = K @ Q^T          (kb-outer, queries streamed wide: N up to 512;
                               the two heads of a pair run CONCURRENTLY via
                               64-partition row tiling of the PE array)
  P^T      = exp(0.125*S^T)   (one ACT op per (kb): [128, 2, w] psum->sbuf)
  mask     on DVE             (always first 128 query cols; data-dependent
                               per core parity: tri/zero vs ones/tri)
  Y^T[d,q] = VN^T @ P^T       (transposed AV: lhsT=VN [k,65], rhs=P [k,q],
                               accumulated over kb in PSUM; row 64 = sums)
  Yn^T     = Y^T * (1/s)      (DVE recip + GPSIMD partition_broadcast + DVE
                               mult straight into YT, the out-proj lhsT)
  Z        = YT.T @ w_out + ones x b_out  (bias as a K=1 matmul; copies DVE)

Emission interleaves proj(pb+1) with attn(pb) so PE projection work fills
the ACT-bound attention window.
"""

import math
import os
import numpy as np

B, T, C = 4, 2048, 1024
H, D = 16, 64
N_CORES = 8
P = 128
QB = 8  # local query blocks per core (of 128 rows)
KB = 16  # key blocks per sequence
PAIRS = 8  # head pairs

_COMPILED = None
LAST_EXEC_NS = None

dbg_stage = os.environ.get("KERNEL_DEBUG_STAGE", "")


def _get_mybir():
    import concourse.mybir as mybir
    return mybir


def split_sync_waits(nc):
    mybir = _get_mybir()
    # This walrus build rejects instructions carrying more than one sync
    # wait (or update).  Split the extras onto NOP carriers: waits go on
    # NOPs inserted before the instruction (same engine, so they gate it),
    # updates onto NOPs after it (fire once it has completed).
    uid = [0]

    def carrier(engine, wait=None, update=None):
        uid[0] += 1
        n = mybir.InstNoOp(
            name=f"I-syncsplit-{uid[0]}",
            opcode="NoOp",
            ins=[],
            outs=[],
            sync_info=mybir.SyncInfo(
                on_wait=[wait] if wait else [],
                on_update=[update] if update else [],
            ),
        )
        n.engine = engine
        return n

    debug = os.environ.get("SYNC_SPLIT_DEBUG")
    for f in nc.m.functions:
        for blk in f.blocks:
            out = []
            changed = False
            for inst in blk.instructions:
                si = inst.sync_info
                if si is None or (
                    len(si.on_wait) <= 1 and len(si.on_update) <= 1
                ):
                    out.append(inst)
                    continue
                if debug:
                    print(
                        f"SPLIT {inst.opcode} {inst.name} eng={inst.engine} "
                        f"waits={len(si.on_wait)} upds={len(si.on_update)}"
                    )
                changed = True
                waits = list(si.on_wait)
                updates = list(si.on_update)
                for w in waits[1:]:
                    out.append(carrier(inst.engine, wait=w))
                inst.sync_info = mybir.SyncInfo(
                    on_wait=waits[:1], on_update=updates[:1]
                )
                out.append(inst)
                for u in updates[1:]:
                    out.append(carrier(inst.engine, update=u))
            if changed:
                blk.instructions = out


def _build(reps=1):
    import concourse.bass as bass
    import concourse.tile as tile
    import concourse.mybir as mybir
    from contextlib import ExitStack, nullcontext

    f32 = mybir.dt.float32
    f16 = mybir.dt.float16
    AF = mybir.ActivationFunctionType

    nc = bass.Bass(
        "TRN2", target_bir_lowering=False, debug=False, num_devices=N_CORES
    )

    xT_d = nc.dram_tensor("xt", [C, T], f16, kind="ExternalInput").ap()
    xqT_d = nc.dram_tensor("xqt", [C, QB * P], f16, kind="ExternalInput").ap()
    wqkv_d = nc.dram_tensor("wqkv", [16, P, 8, P], f16, kind="ExternalInput").ap()
    wv_d = nc.dram_tensor("wv", [P, 8, C], f16, kind="ExternalInput").ap()
    wout_d = nc.dram_tensor("wout", [P, 8, C], f16, kind="ExternalInput").ap()
    bqkv_d = nc.dram_tensor("bqkv", [P, 24], f32, kind="ExternalInput").ap()
    bvb_d = nc.dram_tensor("bvb", [P, C], f16, kind="ExternalInput").ap()
    bout_d = nc.dram_tensor("bout", [1, C], f16, kind="ExternalInput").ap()
    mskD_d = nc.dram_tensor("mskD", [P, 2, 2, P], f16, kind="ExternalInput").ap()
    ones_d = nc.dram_tensor("ones1", [1, P], f16, kind="ExternalInput").ap()
    yr_d = nc.dram_tensor("yr", [P, QB, C], f32, kind="ExternalOutput").ap()
    dbg_d = {}
    if dbg_stage in ("proj", "vn", "attn"):
        dbg_d["kt"] = nc.dram_tensor(
            "kt", [P, PAIRS, T], f16, kind="ExternalOutput"
        ).ap()
        dbg_d["qt"] = nc.dram_tensor(
            "qt", [P, PAIRS, QB * P], f16, kind="ExternalOutput"
        ).ap()
    if dbg_stage in ("vn", "attn"):
        dbg_d["vn"] = nc.dram_tensor(
            "vn", [P, PAIRS, KB, 2, 65], f16, kind="ExternalOutput"
        ).ap()
    if dbg_stage == "attn":
        dbg_d["yt"] = nc.dram_tensor(
            "yt", [P, 8, QB * P], f16, kind="ExternalOutput"
        ).ap()

    with tile.TileContext(nc) as tc, (
        tc.For_i(0, reps) if reps > 1 else nullcontext()
    ), ExitStack() as ctx:
        persist = ctx.enter_context(tc.tile_pool(name="persist", bufs=1))
        KT = persist.tile([P, PAIRS, T], f16)  # K^T + bias, head pairs stacked
        QT = persist.tile([P, PAIRS, QB * P], f16)
        YT = persist.tile([P, 8, QB * P], f16)  # normalized attn out, [c, tq]
        VN = persist.tile([P, PAIRS, KB, 2, 65], f16)  # [V_h | 1], natural
        WO = persist.tile([P, 8, C], f16)
        WV = persist.tile([P, 8, C], f16)
        mskD = persist.tile([P, 2, 2, P], f16)
        bqs = persist.tile([P, 24], f32)
        bvb = persist.tile([P, C], f16)
        on1 = persist.tile([1, P], f16)
        bo = persist.tile([1, C], f16)

        nc.sync.dma_start(out=mskD, in_=mskD_d)
        nc.sync.dma_start(out=bqs, in_=bqkv_d)
        nc.sync.dma_start(out=bvb, in_=bvb_d)
        nc.sync.dma_start(out=on1, in_=ones_d)
        nc.sync.dma_start(out=bo, in_=bout_d)
        nc.sync.dma_start(out=WO, in_=wout_d)
        nc.sync.dma_start(out=WV, in_=wv_d)

        nc.vector.memset(VN[:, :, :, :, 64:65], 1.0)

        # attention pools (outlive the proj scope)
        ptpool = ctx.enter_context(tc.tile_pool(name="pt", bufs=2))
        rpool = ctx.enter_context(tc.tile_pool(name="r", bufs=2))
        psSpool = ctx.enter_context(
            tc.tile_pool(name="psS", bufs=2, space="PSUM")
        )
        psYpool = ctx.enter_context(
            tc.tile_pool(name="psY", bufs=1, space="PSUM")
        )
        wpool = ctx.enter_context(tc.tile_pool(name="w", bufs=2))

        def attn_chunk(pb, chunk):
            # software-pipelined: AV(kb) is emitted after scores(kb+1) so the
            # in-order PE queue is never head-of-line blocked behind exp(kb)
            nkb = 8 * (chunk + 1)
            psY = [
                psYpool.tile([P, 512], f32, tag=f"psY{h}", name=f"psY{h}")
                for h in range(2)
            ]
            pending = None  # (pt, kb, ql, w)

            def emit_av(p):
                pt, kb, ql, w = p
                for h in range(2):
                    nc.tensor.matmul(
                        psY[h][0:65, ql : ql + w],
                        lhsT=VN[:, pb, kb, h],
                        rhs=pt[:, h, :w],
                        start=(kb == 0),
                        stop=(kb == nkb - 1),
                        skip_group_check=True,
                    )

            for kb in range(nkb):
                i0 = max(kb // 2, 4 * chunk)
                w = (4 * (chunk + 1) - i0) * P
                q0 = i0 * P
                ql = q0 - 512 * chunk
                psS = psSpool.tile([P, 2, 512], f32, tag="psS", name="psS")
                for h in range(2):
                    nc.tensor.matmul(
                        psS[:, h, :w],
                        lhsT=KT[64 * h : 64 * h + 64, pb, kb * P : (kb + 1) * P],
                        rhs=QT[64 * h : 64 * h + 64, pb, q0 : q0 + w],
                        start=True,
                        stop=True,
                    )
                pt = ptpool.tile([P, 2, 512], f16, tag="pt")
                nc.scalar.activation(
                    pt[:, :, :w], psS[:, :, :w], AF.Exp, scale=0.125
                )
                if chunk == 0 or kb >= 8:
                    nc.vector.tensor_mul(
                        out=pt[:, :, 0:P],
                        in0=pt[:, :, 0:P],
                        in1=mskD[:, kb % 2],
                    )
                if pending is not None:
                    emit_av(pending)
                pending = (pt, kb, ql, w)
            emit_av(pending)
            for h in range(2):
                r = rpool.tile([32, 512], f16, tag=f"r{h}")
                rb = rpool.tile([32, 512], f16, tag=f"rb{h}")
                with nc.allow_low_precision(reason="softmax recip in f16"):
                    nc.vector.reciprocal(r[0:1], psY[h][64:65, :])
                nc.vector.stream_shuffle(rb, r, [0] * 32)
                yto = YT[:, pb, chunk * 512 : (chunk + 1) * 512]
                for pq in range(2):
                    nc.vector.tensor_mul(
                        out=yto[64 * h + 32 * pq : 64 * h + 32 * pq + 32],
                        in0=psY[h][32 * pq : 32 * pq + 32],
                        in1=rb,
                    )

        xT_v = xT_d.rearrange("(cb p) t -> p cb t", p=P)
        xqT_v = xqT_d.rearrange("(cb p) t -> p cb t", p=P)

        with (
            tc.tile_pool(name="xt", bufs=1) as xtpool,
            tc.tile_pool(name="psproj", bufs=2, space="PSUM") as pspool,
        ):
            XT = xtpool.tile([P, 8, T], f16)
            XQT = xtpool.tile([P, 8, QB * P], f16)
            for cb in range(8):
                nc.sync.dma_start(out=XT[:, cb], in_=xT_v[:, cb])
            for cb in range(8):
                nc.sync.dma_start(out=XQT[:, cb], in_=xqT_v[:, cb])

            def proj_kq(pb):
                # K^T: j-block 8+pb over full T; Q^T: j-block pb over own rows
                wt = wpool.tile([P, 8, P], f16, tag="w")
                nc.sync.dma_start(out=wt, in_=wqkv_d[8 + pb])
                for t4 in range(4):
                    ps = pspool.tile([P, 512], f32, tag="proj")
                    for cb in range(8):
                        nc.tensor.matmul(
                            ps,
                            lhsT=wt[:, cb],
                            rhs=XT[:, cb, 512 * t4 : 512 * (t4 + 1)],
                            start=(cb == 0),
                            stop=(cb == 7),
                        )
                    nc.vector.tensor_scalar_add(
                        out=KT[:, pb, 512 * t4 : 512 * (t4 + 1)],
                        in0=ps,
                        scalar1=bqs[:, 8 + pb : 9 + pb],
                    )
                wt = wpool.tile([P, 8, P], f16, tag="w")
                nc.sync.dma_start(out=wt, in_=wqkv_d[pb])
                for t2 in range(2):
                    ps = pspool.tile([P, 512], f32, tag="proj")
                    for cb in range(8):
                        nc.tensor.matmul(
                            ps,
                            lhsT=wt[:, cb],
                            rhs=XQT[:, cb, 512 * t2 : 512 * (t2 + 1)],
                            start=(cb == 0),
                            stop=(cb == 7),
                        )
                    nc.vector.tensor_scalar_add(
                        out=QT[:, pb, 512 * t2 : 512 * (t2 + 1)],
                        in0=ps,
                        scalar1=bqs[:, pb : pb + 1],
                    )

            def proj_v(half):
                # V natural: out [t-block, 512 v-cols] = pairs 4*half..4*half+3
                bslice = bvb[:, 512 * half : 512 * (half + 1)].rearrange(
                    "p (a b c) -> p a b c", b=2, c=64
                )
                for tb in range(KB):
                    ps = pspool.tile([P, 512], f32, tag="proj")
                    for cb in range(8):
                        nc.tensor.matmul(
                            ps,
                            lhsT=XT[:, cb, tb * P : (tb + 1) * P],
                            rhs=WV[:, cb, 512 * half : 512 * (half + 1)],
                            start=(cb == 0),
                            stop=(cb == 7),
                        )
                    nc.vector.tensor_add(
                        out=VN[:, 4 * half : 4 * half + 4, tb, :, 0:64],
                        in0=ps.rearrange("p (a b c) -> p a b c", b=2, c=64),
                        in1=bslice,
                    )

            attn_on = dbg_stage in ("", "attn", "full")
            proj_kq(0)
            proj_v(0)
            for pb in range(PAIRS):
                if pb + 1 < PAIRS:
                    proj_kq(pb + 1)
                if pb == 2:
                    proj_v(1)
                if attn_on:
                    attn_chunk(pb, 0)

        if "kt" in dbg_d:
            nc.sync.dma_start(out=dbg_d["kt"], in_=KT)
            nc.sync.dma_start(out=dbg_d["qt"], in_=QT)
        if "vn" in dbg_d:
            nc.sync.dma_start(out=dbg_d["vn"], in_=VN)
        if dbg_stage:
            # debug stages: make sure yr is written so outputs exist
            nc.vector.memset(YT[:1, 0, :1], 0.0)
        # chunk-1 attention interleaved with the output projection: out-proj
        # PE work fills the ACT-bound attention window
        with (
            tc.tile_pool(name="z", bufs=2) as zpool,
            tc.tile_pool(name="psZ", bufs=2, space="PSUM") as psZpool,
        ):

            def outproj(i):
                zt = zpool.tile([P, C], f32, tag="z")
                for nc2 in range(2):
                    ps = psZpool.tile([P, 512], f32, tag="z")
                    for cb in range(8):
                        nc.tensor.matmul(
                            ps,
                            lhsT=YT[:, cb, i * P : (i + 1) * P],
                            rhs=WO[:, cb, 512 * nc2 : 512 * (nc2 + 1)],
                            start=(cb == 0),
                            stop=False,
                        )
                    nc.tensor.matmul(
                        ps,
                        lhsT=on1,
                        rhs=bo[:, 512 * nc2 : 512 * (nc2 + 1)],
                        start=False,
                        stop=True,
                    )
                    nc.vector.tensor_copy(
                        out=zt[:, 512 * nc2 : 512 * (nc2 + 1)], in_=ps
                    )
                nc.sync.dma_start(out=yr_d[:, i], in_=zt)

            if attn_on:
                for pb in range(PAIRS):
                    attn_chunk(pb, 1)
                    if pb % 2 == 1:
                        outproj(pb // 2)
                for i in range(4, 8):
                    outproj(i)

        if "yt" in dbg_d:
            nc.sync.dma_start(out=dbg_d["yt"], in_=YT)

    split_sync_waits(nc)
    return nc


def _host_inputs(x, w_qkv, b_qkv, w_out, b_out):
    x = np.asarray(x, dtype=np.float32)
    w_qkv = np.asarray(w_qkv, dtype=np.float32)
    b_qkv = np.asarray(b_qkv, dtype=np.float32)
    w_out = np.asarray(w_out, dtype=np.float32)
    b_out = np.asarray(b_out, dtype=np.float32)

    wqkv_r = np.ascontiguousarray(
        w_qkv.reshape(8, P, 24, P).transpose(2, 1, 0, 3)[0:16]
    ).astype(np.float16)
    wv_r = np.ascontiguousarray(
        w_qkv[:, 2 * C : 3 * C].reshape(8, P, C).transpose(1, 0, 2)
    ).astype(np.float16)
    wout_r = np.ascontiguousarray(
        w_out.reshape(8, P, C).transpose(1, 0, 2)
    ).astype(np.float16)
    bqkv_r = np.ascontiguousarray(b_qkv.reshape(24, P).T)
    bvb_r = np.ascontiguousarray(
        np.broadcast_to(b_qkv[2 * C : 3 * C], (P, C))
    ).astype(np.float16)
    bout_r = b_out.reshape(1, C).astype(np.float16)
    tri = np.triu(np.ones((P, P), dtype=np.float16))  # [k, q]: k <= q
    zer = np.zeros((P, P), dtype=np.float16)
    one = np.ones((P, P), dtype=np.float16)
    ones1 = np.ones((1, P), dtype=np.float16)

    in_maps = []
    for c in range(N_CORES):
        b, par = c // 2, c % 2
        xb = x[b]
        xT = np.ascontiguousarray(xb.T).astype(np.float16)
        qg = [2 * i + par for i in range(QB)]
        xq = np.concatenate([xb[g * P : (g + 1) * P] for g in qg], axis=0)
        xqT = np.ascontiguousarray(xq.T).astype(np.float16)
        m0, m1 = (tri, zer) if par == 0 else (one, tri)  # [even kb, odd kb]
        mskD = np.ascontiguousarray(
            np.stack([np.stack([m0, m0]), np.stack([m1, m1])]).transpose(2, 0, 1, 3)
        )  # [k, kb%2, h, q]
        in_maps.append(
            {
                "xt": xT,
                "xqt": xqT,
                "wqkv": wqkv_r,
                "wv": wv_r,
                "wout": wout_r,
                "bqkv": bqkv_r,
                "bvb": bvb_r,
                "bout": bout_r,
                "mskD": mskD,
                "ones1": ones1,
            }
        )
    return in_maps


def kernel(x, w_qkv, b_qkv, w_out, b_out, trace=False):
    global _COMPILED, LAST_EXEC_NS
    from concourse import bass_utils

    if _COMPILED is None:
        _COMPILED = _build()
    nc = _COMPILED

    in_maps = _host_inputs(x, w_qkv, b_qkv, w_out, b_out)
    res = bass_utils.run_bass_kernel_spmd(
        nc, in_maps, core_ids=list(range(N_CORES)), trace=trace
    )
    LAST_EXEC_NS = res.exec_time_ns

    y = np.empty((B, T, C), dtype=np.float32)
    for c in range(N_CORES):
        b, par = c // 2, c % 2
        yl = res.results[c]["yr"].transpose(1, 0, 2)  # [QB, P, C]
        for i in range(QB):
            g = 2 * i + par
            y[b, g * P : (g + 1) * P] = yl[i]
    return y
